# revision 15
# baseline (speedup 1.0000x reference)
"""Trainium2 Bass kernel for nn_Encoders (2-layer shared-weight transformer encoder).

Sharding: 8 cores; pair (2b, 2b+1) handles batch b.  Within a pair:
  - attention heads split 4+4 (tensor-parallel), partial out-proj summed by AllReduce
  - FFN split along DFF 1024+1024, partial down-proj summed by AllReduce
All on-chip tensors live in "transposed" layout [feature, token] so every matmul
contracts over the partition axis; LN stats use an all-ones matmul (broadcast sums).

Softmax is the reference's joint 2D softmax over (S,S): exp((logits + mask*NEG)/8)
normalized by the global per-(b,h) sum Z.  Masked entries underflow to exactly 0,
matching the f32 reference.  Z is accumulated free via activation accum_out.

Matmul inputs are float32r (full-rate PE, fp32 storage).  walrus requires every
f32r matmul operand to be produced as f32r, so compute producers write f32r tiles
and DVE/ACT consumers read them through a bitcast back to f32.
"""

import sys

sys.path.insert(0, "/opt/trn_rl_repo")

import numpy as np
import ml_dtypes

import concourse.bass as bass
import concourse.mybir as mybir
import concourse.tile as tile
from concourse import bacc
from concourse.bass_utils import run_bass_kernel_spmd

F32 = mybir.dt.float32
F32R = mybir.dt.float32r
BF16 = mybir.dt.bfloat16
AF = mybir.ActivationFunctionType
OP = mybir.AluOpType
AX = mybir.AxisListType

B, S, D, H, DFF = 4, 1024, 512, 8, 2048
DEPTH = D // H  # 64
NEG = -1.0e9
EPS = 1e-9
N_CORES = 8
GROUPS = [[0, 1], [2, 3], [4, 5], [6, 7]]

HH = H // 2          # heads per core: 4
DH = HH * DEPTH      # own head dims: 256
DFO = DFF // 2       # own dff: 1024

KD = D // 128     # 4 k-tiles over D
MQ = DH // 128    # 2 col-tiles over own head dims
IT = S // 128     # 8 i tiles
JC = S // 512     # 2 j chunks
KF = DFO // 128   # 8 dff tiles


def _rd(ap):
    """read a float32r tile as plain float32 (DVE/ACT consumers)"""
    return ap.bitcast(F32)


def _wr(ap):
    """write-view of a float32 tile as float32r (rounding producer)"""
    return ap.bitcast(F32R)


def build():
    nc = bacc.Bacc("TRN2", target_bir_lowering=False, debug=False,
                   num_devices=N_CORES)

    def din(name, shape, dt=F32):
        return nc.dram_tensor(name, shape, dt, kind="ExternalInput").ap()

    xT = din("xT", [D, S], F32R)
    maskp = din("maskp", [S, S], BF16)          # mask * (-1e9), bf16
    ident = din("ident", [128, 128], BF16)      # identity matrix
    wq = din("wq", [D, DH], F32R)
    wk = din("wk", [D, DH], F32R)
    wv = din("wv", [D, DH], F32R)
    wo = din("wo", [DH, D], F32R)
    w1 = din("w1", [D, DFO], F32R)
    w2 = din("w2", [DFO, D], F32R)
    bq = din("bq", [128, DH // 128])
    bk = din("bk", [128, DH // 128])
    bv = din("bv", [1, DH], F32R)
    bo = din("bo", [128, D // 128])   # already halved on host
    b1 = din("b1", [128, DFO // 128])
    b2 = din("b2", [128, D // 128])   # already halved on host
    g1 = din("g1", [128, D // 128])
    be1 = din("be1", [128, D // 128])
    g2 = din("g2", [128, D // 128])
    be2 = din("be2", [128, D // 128])
    hout = nc.dram_tensor("hout", [D, S], F32, kind="ExternalOutput").ap()

    with tile.TileContext(nc) as tc:
        with (
            tc.tile_pool(name="const", bufs=1) as const,
            tc.tile_pool(name="state", bufs=1) as state,
            tc.tile_pool(name="scr", bufs=1) as scr,
            tc.tile_pool(name="psum", bufs=8, space="PSUM") as psum,
            tc.tile_pool(name="dram", bufs=1, space="DRAM") as dram,
        ):
            # ---- constants / weights ----
            def loadw(name, src, kt, m):
                t = const.tile([128, kt, m], F32R, name=name, tag=name)
                nc.sync.dma_start(out=t, in_=src.rearrange("(k p) m -> p k m", p=128))
                return t

            wq_sb = loadw("wq_sb", wq, KD, DH)
            wk_sb = loadw("wk_sb", wk, KD, DH)
            wv_sb = loadw("wv_sb", wv, KD, DH)
            wo_sb = loadw("wo_sb", wo, MQ, D)
            w1_sb = loadw("w1_sb", w1, KD, DFO)
            w2_sb = loadw("w2_sb", w2, KF, D)

            maskp_sb = const.tile([128, IT, S], BF16, name="maskp_sb", tag="maskp_sb")
            nc.sync.dma_start(out=maskp_sb, in_=maskp.rearrange("(i p) j -> p i j", p=128))
            id_sb = const.tile([128, 128], BF16, name="id_sb", tag="id_sb")
            nc.sync.dma_start(out=id_sb, in_=ident)

            def loadsm(name, src, dt=F32):
                t = const.tile(list(src.shape), dt, name=name, tag=name)
                nc.sync.dma_start(out=t, in_=src)
                return t

            bq_sb = loadsm("bq_sb", bq)
            bk_sb = loadsm("bk_sb", bk)
            bv_sb = loadsm("bv_sb", bv, F32R)
            bo_sb = loadsm("bo_sb", bo)
            b1_sb = loadsm("b1_sb", b1)
            b2_sb = loadsm("b2_sb", b2)
            g1_sb = loadsm("g1_sb", g1)
            be1_sb = loadsm("be1_sb", be1)
            g2_sb = loadsm("g2_sb", g2)
            be2_sb = loadsm("be2_sb", be2)

            ones_sq = const.tile([128, 128], F32R, name="ones_sq", tag="ones_sq")
            nc.vector.memset(_rd(ones_sq), 1.0)
            ones_f32 = const.tile([128, 128], F32, name="ones_f32", tag="ones_f32")
            nc.vector.memset(ones_f32, 1.0)
            eps_sb = const.tile([128, 1], F32, name="eps_sb", tag="eps_sb")
            nc.vector.memset(eps_sb, EPS)

            # ---- initial h (= x, transposed) ----
            h_cur = state.tile([128, KD, S], F32R, name="h0", tag="h", bufs=2)
            nc.sync.dma_start(out=h_cur, in_=xT.rearrange("(k p) s -> p k s", p=128))

            def layernorm(z, g_sb, be_sb, out_name, out_tag=None, out_bufs=1):
                """z: f32 [128, KD, S] tile. Normalizes over the partition-tiled
                feature axis via all-ones matmul sums. Returns f32r tile."""
                s1 = [psum.tile([128, 512], F32, name=f"s1_{out_name}_{jc}", tag="ps")
                      for jc in range(JC)]
                s2 = [psum.tile([128, 512], F32, name=f"s2_{out_name}_{jc}", tag="ps")
                      for jc in range(JC)]
                for k in range(KD):
                    for jc in range(JC):
                        sl = slice(jc * 512, (jc + 1) * 512)
                        sqc = scr.tile([128, 512], F32R, name=f"sq_{out_name}_{k}_{jc}",
                                       tag="e", bufs=4)
                        nc.scalar.activation(out=sqc, in_=z[:, k, sl], func=AF.Square)
                        nc.tensor.matmul(s1[jc], lhsT=ones_sq, rhs=_wr(z[:, k, sl]),
                                         start=(k == 0), stop=(k == KD - 1))
                        nc.tensor.matmul(s2[jc], lhsT=ones_sq, rhs=sqc,
                                         start=(k == 0), stop=(k == KD - 1))
                mean = scr.tile([128, S], F32, name=f"mean_{out_name}", tag="mean", bufs=1)
                rstd = scr.tile([128, S], F32, name=f"rstd_{out_name}", tag="rstd", bufs=1)
                for jc in range(JC):
                    sl = slice(jc * 512, (jc + 1) * 512)
                    nc.vector.tensor_scalar(out=mean[:, sl], in0=s1[jc], scalar1=1.0 / D,
                                            scalar2=None, op0=OP.mult)
                    msq = scr.tile([128, 512], F32, name=f"msq_{out_name}_{jc}",
                                   tag="e", bufs=4)
                    nc.vector.tensor_tensor(out=msq, in0=mean[:, sl], in1=mean[:, sl],
                                            op=OP.mult)
                    var = scr.tile([128, 512], F32, name=f"var_{out_name}_{jc}",
                                   tag="e", bufs=4)
                    nc.vector.scalar_tensor_tensor(out=var, in0=s2[jc], scalar=1.0 / D,
                                                   in1=msq, op0=OP.mult, op1=OP.subtract)
                    nc.scalar.activation(out=var, in_=var, func=AF.Sqrt, bias=eps_sb[:, 0:1])
                    nc.vector.reciprocal(out=rstd[:, sl], in_=var)
                hn = state.tile([128, KD, S], F32R, name=out_name,
                                tag=out_tag or out_name[:2], bufs=out_bufs)
                for k in range(KD):
                    for jc in range(JC):
                        sl = slice(jc * 512, (jc + 1) * 512)
                        t = scr.tile([128, 512], F32, name=f"t_{out_name}_{k}_{jc}",
                                     tag="e", bufs=4)
                        nc.vector.tensor_tensor(out=t, in0=z[:, k, sl], in1=mean[:, sl],
                                                op=OP.subtract)
                        nc.vector.tensor_tensor(out=t, in0=t, in1=rstd[:, sl], op=OP.mult)
                        nc.vector.tensor_scalar(out=hn[:, k, sl], in0=t,
                                                scalar1=g_sb[:, k:k + 1],
                                                scalar2=be_sb[:, k:k + 1],
                                                op0=OP.mult, op1=OP.add)
                return hn

            for ly in range(2):
                # ---- A: qkv projections ----
                qT = state.tile([128, MQ, S], F32R, name=f"qT_{ly}", tag="qT", bufs=1)
                kT = state.tile([128, MQ, S], F32R, name=f"kT_{ly}", tag="kT", bufs=1)
                for m in range(MQ):
                    for sc in range(JC):
                        ssl = slice(sc * 512, (sc + 1) * 512)
                        q_ps = psum.tile([128, 512], F32, name=f"q_ps_{ly}_{m}_{sc}", tag="ps")
                        k_ps = psum.tile([128, 512], F32, name=f"k_ps_{ly}_{m}_{sc}", tag="ps")
                        for k in range(KD):
                            nc.tensor.matmul(q_ps, lhsT=wq_sb[:, k, m * 128:(m + 1) * 128],
                                             rhs=h_cur[:, k, ssl],
                                             start=(k == 0), stop=(k == KD - 1))
                        for k in range(KD):
                            nc.tensor.matmul(k_ps, lhsT=wk_sb[:, k, m * 128:(m + 1) * 128],
                                             rhs=h_cur[:, k, ssl],
                                             start=(k == 0), stop=(k == KD - 1))
                        nc.scalar.activation(out=qT[:, m, ssl], in_=q_ps,
                                             func=AF.Identity, bias=bq_sb[:, m:m + 1])
                        nc.scalar.activation(out=kT[:, m, ssl], in_=k_ps,
                                             func=AF.Identity, bias=bk_sb[:, m:m + 1])
                v_sb = state.tile([128, IT, DH], F32R, name=f"v_{ly}", tag="v", bufs=1)
                for it in range(IT):
                    v_ps = psum.tile([128, DH], F32, name=f"v_ps_{ly}_{it}", tag="ps")
                    for k in range(KD):
                        nc.tensor.matmul(v_ps, lhsT=h_cur[:, k, it * 128:(it + 1) * 128],
                                         rhs=wv_sb[:, k, :],
                                         start=(k == 0), stop=False)
                    nc.tensor.matmul(v_ps, lhsT=ones_sq[0:1, 0:128],
                                     rhs=bv_sb, start=False, stop=True)
                    nc.vector.tensor_copy(out=v_sb[:, it, :], in_=v_ps)

                # ---- B: attention (4 own heads) ----
                outT = state.tile([128, MQ, S], F32R, name=f"outT_{ly}", tag="outT", bufs=1)
                for hh in range(HH):
                    hp, hr = hh // 2, hh % 2
                    pb = 64 * hr
                    q_h = qT[pb:pb + 64, hp, :]
                    k_h = kT[pb:pb + 64, hp, :]
                    acc = scr.tile([128, IT * JC], F32, name=f"acc_{ly}_{hh}",
                                   tag="acc", bufs=2)
                    o_ps = [psum.tile([128, 512], F32, name=f"o_ps_{ly}_{hh}_{jc}", tag="ps")
                            for jc in range(JC)]
                    for jc in range(JC):
                        jsl = slice(jc * 512, (jc + 1) * 512)
                        for it in range(IT):
                            l_ps = psum.tile([128, 512], F32,
                                             name=f"l_ps_{ly}_{hh}_{jc}_{it}", tag="ps")
                            nc.tensor.matmul(l_ps, lhsT=id_sb,
                                             rhs=maskp_sb[:, it, jsl],
                                             start=True, stop=False)
                            nc.tensor.matmul(l_ps, lhsT=q_h[:, it * 128:(it + 1) * 128],
                                             rhs=k_h[:, jsl], start=False, stop=True)
                            e = scr.tile([128, 512], F32R, name=f"e_{ly}_{hh}_{jc}_{it}",
                                         tag="e", bufs=4)
                            ci = jc * IT + it
                            nc.scalar.activation(out=e, in_=l_ps, func=AF.Exp, scale=0.125,
                                                 accum_out=acc[:, ci:ci + 1])
                            nc.tensor.matmul(o_ps[jc][0:64, :],
                                             lhsT=v_sb[:, it, hh * 64:hh * 64 + 64],
                                             rhs=e, start=(it == 0), stop=(it == IT - 1))
                    rsum = scr.tile([128, 1], F32, name=f"rsum_{ly}_{hh}", tag="rsum", bufs=2)
                    nc.vector.reduce_sum(out=rsum, in_=acc, axis=AX.X)
                    z_ps = psum.tile([128, 1], F32, name=f"z_ps_{ly}_{hh}", tag="ps")
                    nc.tensor.matmul(z_ps, lhsT=ones_f32, rhs=rsum, start=True, stop=True)
                    zinv = scr.tile([128, 1], F32, name=f"zinv_{ly}_{hh}", tag="zinv", bufs=2)
                    nc.vector.reciprocal(out=zinv, in_=z_ps)
                    for jc in range(JC):
                        nc.scalar.activation(out=outT[pb:pb + 64, hp, jc * 512:(jc + 1) * 512],
                                             in_=o_ps[jc][0:64, :], func=AF.Identity,
                                             scale=zinv[0:64, 0:1])

                # ---- C: out-projection + AllReduce ----
                cc1_in = dram.tile([D, S], F32, name=f"cc1_in_{ly}", tag="cc1_in")
                cc1_out = dram.tile([D, S], F32, name=f"cc1_out_{ly}", tag="cc1_out")
                cst = state.tile([128, KD, S], F32, name=f"cst1_{ly}", tag="cc", bufs=1)
                for dt_ in range(KD):
                    for jc in range(JC):
                        ap_ps = psum.tile([128, 512], F32, name=f"ap_ps_{ly}_{dt_}_{jc}",
                                          tag="ps")
                        for k in range(MQ):
                            nc.tensor.matmul(ap_ps, lhsT=wo_sb[:, k, dt_ * 128:(dt_ + 1) * 128],
                                             rhs=outT[:, k, jc * 512:(jc + 1) * 512],
                                             start=(k == 0), stop=(k == MQ - 1))
                        nc.vector.tensor_scalar(out=cst[:, dt_, jc * 512:(jc + 1) * 512],
                                                in0=ap_ps, scalar1=bo_sb[:, dt_:dt_ + 1],
                                                scalar2=None, op0=OP.add)
                nc.sync.dma_start(out=cc1_in.rearrange("(k p) s -> p k s", p=128), in_=cst)
                nc.gpsimd.collective_compute("AllReduce", OP.add, replica_groups=GROUPS,
                                             ins=[cc1_in.opt()], outs=[cc1_out.opt()])
                apT = state.tile([128, KD, S], F32, name=f"apT_{ly}", tag="cc", bufs=1)
                nc.sync.dma_start(out=apT, in_=cc1_out.rearrange("(k p) s -> p k s", p=128))

                # ---- D: residual + LN1 (residual written in place, f32r-rounded) ----
                for k in range(KD):
                    nc.vector.tensor_tensor(out=_wr(apT[:, k, :]), in0=_rd(h_cur[:, k, :]),
                                            in1=apT[:, k, :], op=OP.add)
                h1 = layernorm(apT, g1_sb, be1_sb, f"h1_{ly}")

                # ---- E: FFN (own dff half) + AllReduce ----
                cc2_in = dram.tile([D, S], F32, name=f"cc2_in_{ly}", tag="cc2_in")
                cc2_out = dram.tile([D, S], F32, name=f"cc2_out_{ly}", tag="cc2_out")
                cst2 = state.tile([128, KD, S], F32, name=f"cst2_{ly}", tag="cc", bufs=1)
                for jc in range(JC):
                    jsl = slice(jc * 512, (jc + 1) * 512)
                    g_ps = [psum.tile([128, 512], F32, name=f"g_ps_{ly}_{jc}_{d}", tag="ps")
                            for d in range(KD)]
                    for ft in range(KF):
                        f_ps = psum.tile([128, 512], F32, name=f"f_ps_{ly}_{jc}_{ft}",
                                         tag="ps")
                        for k in range(KD):
                            nc.tensor.matmul(f_ps, lhsT=w1_sb[:, k, ft * 128:(ft + 1) * 128],
                                             rhs=h1[:, k, jsl],
                                             start=(k == 0), stop=(k == KD - 1))
                        fr = scr.tile([128, 512], F32R, name=f"fr_{ly}_{jc}_{ft}",
                                      tag="fr", bufs=2)
                        nc.scalar.activation(out=fr, in_=f_ps, func=AF.Relu,
                                             bias=b1_sb[:, ft:ft + 1])
                        for d in range(KD):
                            nc.tensor.matmul(g_ps[d], lhsT=w2_sb[:, ft, d * 128:(d + 1) * 128],
                                             rhs=fr, start=(ft == 0), stop=(ft == KF - 1))
                    for d in range(KD):
                        nc.vector.tensor_scalar(out=cst2[:, d, jsl],
                                                in0=g_ps[d], scalar1=b2_sb[:, d:d + 1],
                                                scalar2=None, op0=OP.add)
                nc.sync.dma_start(out=cc2_in.rearrange("(k p) s -> p k s", p=128), in_=cst2)
                nc.gpsimd.collective_compute("AllReduce", OP.add, replica_groups=GROUPS,
                                             ins=[cc2_in.opt()], outs=[cc2_out.opt()])
                ff = state.tile([128, KD, S], F32, name=f"ff_{ly}", tag="cc", bufs=1)
                nc.sync.dma_start(out=ff, in_=cc2_out.rearrange("(k p) s -> p k s", p=128))

                # ---- F: residual + LN2 -> next h ----
                for k in range(KD):
                    nc.vector.tensor_tensor(out=_wr(ff[:, k, :]), in0=_rd(h1[:, k, :]),
                                            in1=ff[:, k, :], op=OP.add)
                h_cur = layernorm(ff, g2_sb, be2_sb, f"hn_{ly}", out_tag="h", out_bufs=2)

            nc.sync.dma_start(out=hout.rearrange("(k p) s -> p k s", p=128),
                              in_=_rd(h_cur))

    nc.compile()
    return nc


_CACHE = {}


def _prep_inputs(x, mask, Wq, bq, Wk, bk, Wv, bv, Wo, bo, W1, b1, W2, b2,
                 g1, be1, g2, be2):
    f32 = np.float32
    x = np.asarray(x, f32)
    mask = np.asarray(mask, f32)
    ident = np.eye(128, dtype=ml_dtypes.bfloat16)

    def pp(v, cols):  # per-partition layout [128, cols]
        return np.ascontiguousarray(np.asarray(v, f32).reshape(cols, 128).T)

    in_maps = []
    for c in range(N_CORES):
        b, r = c // 2, c % 2
        hs = slice(r * DH, (r + 1) * DH)
        fs = slice(r * DFO, (r + 1) * DFO)
        m = {
            "xT": np.ascontiguousarray(x[b].T),
            "maskp": (mask[b] * NEG).astype(ml_dtypes.bfloat16),
            "ident": ident,
            "wq": np.ascontiguousarray(np.asarray(Wq, f32)[:, hs]),
            "wk": np.ascontiguousarray(np.asarray(Wk, f32)[:, hs]),
            "wv": np.ascontiguousarray(np.asarray(Wv, f32)[:, hs]),
            "wo": np.ascontiguousarray(np.asarray(Wo, f32)[hs, :]),
            "w1": np.ascontiguousarray(np.asarray(W1, f32)[:, fs]),
            "w2": np.ascontiguousarray(np.asarray(W2, f32)[fs, :]),
            "bq": pp(np.asarray(bq, f32)[hs], DH // 128),
            "bk": pp(np.asarray(bk, f32)[hs], DH // 128),
            "bv": np.asarray(bv, f32)[None, hs].copy(),
            "bo": pp(np.asarray(bo, f32) * 0.5, D // 128),
            "b1": pp(np.asarray(b1, f32)[fs], DFO // 128),
            "b2": pp(np.asarray(b2, f32) * 0.5, D // 128),
            "g1": pp(g1, D // 128),
            "be1": pp(be1, D // 128),
            "g2": pp(g2, D // 128),
            "be2": pp(be2, D // 128),
        }
        in_maps.append(m)
    return in_maps


def get_nc():
    if "nc" not in _CACHE:
        _CACHE["nc"] = build()
    return _CACHE["nc"]


def run(in_maps, **kw):
    nc = get_nc()
    return run_bass_kernel_spmd(nc, in_maps, core_ids=list(range(N_CORES)), **kw)


def kernel(**inputs):
    in_maps = _prep_inputs(**inputs)
    res = run(in_maps)
    out = np.empty((B, S, D), np.float32)
    for b in range(B):
        out[b] = res.results[2 * b]["hout"].T
    return out


# revision 25
# speedup vs baseline: 1.5975x; 1.5975x over previous
"""Trainium2 Bass kernel for nn_Encoders (2-layer shared-weight transformer encoder).

Sharding (v3): 8 cores; pair (2b, 2b+1) handles batch b.  Within a pair the
split is along the attention *output* token axis j (the reference's unusual
attention contracts over queries i: out[j,d] = sum_i attn[i,j] v[i,d]):

  - each core computes q and v for ALL tokens/heads (small duplication),
    k only for its own j-half,
  - E = exp((qk^T + mask*NEG)/8) for its own j columns, all heads,
  - attention output, out-projection, residual+LN1, full-DFF FFN,
    residual+LN2 for its own j-half only -- NO partial-sum collectives.

Cross-core data: the joint-softmax denominator Z (per head) is summed with a
32-byte AllReduce, and the layer output h is AllGathered (1MB) at the layer
boundary; the final layer outputs each core's own half directly.

Everything stays in transposed layout [feature, token]; LN stats via all-ones
matmul (broadcast sums).  Matmul inputs are float32r (full-rate PE, fp32
storage): producers write f32r, DVE/ACT consumers read via bitcast to f32.
A ones-column appended to v (via host-built wv_aug/bv_aug) makes the attnV
matmul emit per-column E sums for free -> Z without activation accumulators.
"""

import sys

sys.path.insert(0, "/opt/trn_rl_repo")

import numpy as np
import ml_dtypes

import concourse.bass as bass
import concourse.mybir as mybir
import concourse.tile as tile
from concourse import bacc
from concourse.bass_utils import run_bass_kernel_spmd

F32 = mybir.dt.float32
F32R = mybir.dt.float32r
BF16 = mybir.dt.bfloat16
AF = mybir.ActivationFunctionType
OP = mybir.AluOpType
AX = mybir.AxisListType

B, S, D, H, DFF = 4, 1024, 512, 8, 2048
DEPTH = D // H  # 64
NEG = -1.0e9
EPS = 1e-9
N_CORES = 8
GROUPS = [[0, 1], [2, 3], [4, 5], [6, 7]]

SJ = S // 2       # own token half: 512
KD = D // 128     # 4 k-tiles over D
IT = S // 128     # 8 i tiles
KF = DFF // 128   # 16 dff tiles
VA = 2 * (4 * 65)  # v augmented with a ones column per head: 2 halves x 260


def _rd(ap):
    return ap.bitcast(F32)


def build():
    nc = bacc.Bacc("TRN2", target_bir_lowering=False, debug=False,
                   num_devices=N_CORES)

    def din(name, shape, dt=F32):
        return nc.dram_tensor(name, shape, dt, kind="ExternalInput").ap()

    xT = din("xT", [D, S], F32R)
    maskp = din("maskp", [S, SJ], BF16)         # mask[:, own j] * (-1e9)
    ident = din("ident", [128, 128], BF16)
    wq = din("wq", [D, D], F32R)
    wk = din("wk", [D, D], F32R)
    wva = din("wva", [D, VA], F32R)             # v weights with ones-cols
    wo = din("wo", [D, D], F32R)
    w1 = din("w1", [D, DFF], F32R)
    w2 = din("w2", [DFF, D], F32R)
    bq = din("bq", [128, KD])
    bk = din("bk", [128, KD])
    bva = din("bva", [1, VA], F32R)
    bo = din("bo", [128, KD])
    b1 = din("b1", [128, KF])
    b2 = din("b2", [128, KD])
    g1 = din("g1", [128, KD])
    be1 = din("be1", [128, KD])
    g2 = din("g2", [128, KD])
    be2 = din("be2", [128, KD])
    id8 = din("id8", [8, 8])
    selp = din("selp", [8, 128], F32R)
    hout = nc.dram_tensor("hout", [D, SJ], F32, kind="ExternalOutput").ap()

    with tile.TileContext(nc) as tc:
        with (
            tc.tile_pool(name="const", bufs=1) as const,
            tc.tile_pool(name="state", bufs=1) as state,
            tc.tile_pool(name="scr", bufs=1) as scr,
            tc.tile_pool(name="psum", bufs=8, space="PSUM") as psum,
            tc.tile_pool(name="dram", bufs=1, space="DRAM") as dram,
        ):
            def loadw(name, src, kt, m):
                t = const.tile([128, kt, m], F32R, name=name, tag=name)
                nc.sync.dma_start(out=t, in_=src.rearrange("(k p) m -> p k m", p=128))
                return t

            def loadsm(name, src, dt=F32):
                t = const.tile(list(src.shape), dt, name=name, tag=name)
                nc.sync.dma_start(out=t, in_=src)
                return t

            # load order matters: the first qkv matmuls need x + wq/wk/wva
            h_cur = state.tile([128, KD, S], F32R, name="h0", tag="h", bufs=2)
            nc.sync.dma_start(out=h_cur, in_=xT.rearrange("(k p) s -> p k s", p=128))
            wq_sb = loadw("wq_sb", wq, KD, D)
            wk_sb = loadw("wk_sb", wk, KD, D)
            wva_sb = loadw("wva_sb", wva, KD, VA)
            bq_sb = loadsm("bq_sb", bq)
            bk_sb = loadsm("bk_sb", bk)
            bva_sb = loadsm("bva_sb", bva, F32R)
            ones_sq = const.tile([128, 128], F32R, name="ones_sq", tag="ones_sq")
            nc.vector.memset(_rd(ones_sq), 1.0)
            maskp_sb = const.tile([128, IT, SJ], BF16, name="maskp_sb", tag="maskp_sb")
            nc.sync.dma_start(out=maskp_sb, in_=maskp.rearrange("(i p) j -> p i j", p=128))
            id_sb = const.tile([128, 128], BF16, name="id_sb", tag="id_sb")
            nc.sync.dma_start(out=id_sb, in_=ident)
            wo_sb = loadw("wo_sb", wo, KD, D)
            w1_sb = loadw("w1_sb", w1, KD, DFF)
            bo_sb = loadsm("bo_sb", bo)
            b1_sb = loadsm("b1_sb", b1)
            b2_sb = loadsm("b2_sb", b2)
            g1_sb = loadsm("g1_sb", g1)
            be1_sb = loadsm("be1_sb", be1)
            g2_sb = loadsm("g2_sb", g2)
            be2_sb = loadsm("be2_sb", be2)
            eps_sb = const.tile([128, 1], F32, name="eps_sb", tag="eps_sb")
            nc.vector.memset(eps_sb, EPS)
            id8_sb = loadsm("id8_sb", id8)
            selp_sb = loadsm("selp_sb", selp, F32R)
            w2r = w2.rearrange("(k p) m -> p k m", p=128)

            def layernorm(z, g_sb, be_sb, out_name, out_tile):
                """z: f32r [128, KD, SJ]; writes normalized f32r into out_tile."""
                s1 = psum.tile([128, SJ], F32, name=f"s1_{out_name}", tag="ps")
                s2 = psum.tile([128, SJ], F32, name=f"s2_{out_name}", tag="ps")
                for k in range(KD):
                    sqc = scr.tile([128, SJ], F32R, name=f"sq_{out_name}_{k}",
                                   tag="e", bufs=3)
                    nc.scalar.activation(out=sqc, in_=_rd(z[:, k, :]), func=AF.Square)
                    nc.tensor.matmul(s1, lhsT=ones_sq, rhs=z[:, k, :],
                                     start=(k == 0), stop=(k == KD - 1))
                    nc.tensor.matmul(s2, lhsT=ones_sq, rhs=sqc,
                                     start=(k == 0), stop=(k == KD - 1))
                mean = scr.tile([128, SJ], F32, name=f"mean_{out_name}", tag="mean", bufs=1)
                rstd = scr.tile([128, SJ], F32, name=f"rstd_{out_name}", tag="rstd", bufs=1)
                nc.vector.tensor_scalar(out=mean, in0=s1, scalar1=1.0 / D,
                                        scalar2=None, op0=OP.mult)
                msq = scr.tile([128, SJ], F32, name=f"msq_{out_name}", tag="e", bufs=3)
                nc.vector.tensor_tensor(out=msq, in0=mean, in1=mean, op=OP.mult)
                var = scr.tile([128, SJ], F32, name=f"var_{out_name}", tag="e", bufs=3)
                nc.vector.scalar_tensor_tensor(out=var, in0=s2, scalar=1.0 / D,
                                               in1=msq, op0=OP.mult, op1=OP.subtract)
                nc.scalar.activation(out=var, in_=var, func=AF.Sqrt, bias=eps_sb[:, 0:1])
                nc.vector.reciprocal(out=rstd, in_=var)
                for k in range(KD):
                    t = scr.tile([128, SJ], F32, name=f"t_{out_name}_{k}",
                                 tag="e", bufs=3)
                    nc.vector.tensor_tensor(out=t, in0=_rd(z[:, k, :]), in1=mean,
                                            op=OP.subtract)
                    nc.vector.tensor_tensor(out=t, in0=t, in1=rstd, op=OP.mult)
                    nc.vector.tensor_scalar(out=out_tile[:, k, :], in0=t,
                                            scalar1=g_sb[:, k:k + 1],
                                            scalar2=be_sb[:, k:k + 1],
                                            op0=OP.mult, op1=OP.add)

            for ly in range(2):
                # ---- A: projections. q,v for all tokens; k for own half ----
                qT = state.tile([128, KD, S], F32R, name=f"qT_{ly}", tag="qz", bufs=1)
                kT = state.tile([128, KD, SJ], F32R, name=f"kT_{ly}", tag="kT", bufs=1)
                for m in range(KD):
                    k_ps = psum.tile([128, SJ], F32, name=f"k_ps_{ly}_{m}", tag="ps")
                    for k in range(KD):
                        nc.tensor.matmul(k_ps, lhsT=wk_sb[:, k, m * 128:(m + 1) * 128],
                                         rhs=h_cur[:, k, 0:SJ],
                                         start=(k == 0), stop=(k == KD - 1))
                    nc.scalar.activation(out=kT[:, m, :], in_=k_ps,
                                         func=AF.Identity, bias=bk_sb[:, m:m + 1])
                for m in range(KD):
                    for sc in range(2):
                        ssl = slice(sc * SJ, (sc + 1) * SJ)
                        q_ps = psum.tile([128, SJ], F32, name=f"q_ps_{ly}_{m}_{sc}", tag="ps")
                        for k in range(KD):
                            nc.tensor.matmul(q_ps, lhsT=wq_sb[:, k, m * 128:(m + 1) * 128],
                                             rhs=h_cur[:, k, ssl],
                                             start=(k == 0), stop=(k == KD - 1))
                        nc.scalar.activation(out=qT[:, m, ssl], in_=q_ps,
                                             func=AF.Identity, bias=bq_sb[:, m:m + 1])
                v_sb = state.tile([128, IT, 2, 260], F32R, name=f"v_{ly}", tag="v", bufs=1)
                for it in range(IT):
                    for hf in range(2):
                        v_ps = psum.tile([128, 260], F32, name=f"v_ps_{ly}_{it}_{hf}",
                                         tag="ps")
                        for k in range(KD):
                            nc.tensor.matmul(v_ps,
                                             lhsT=h_cur[:, k, it * 128:(it + 1) * 128],
                                             rhs=wva_sb[:, k, hf * 260:(hf + 1) * 260],
                                             start=(k == 0), stop=False)
                        nc.tensor.matmul(v_ps, lhsT=ones_sq[0:1, 0:128],
                                         rhs=bva_sb[0:1, hf * 260:(hf + 1) * 260],
                                         start=False, stop=True)
                        nc.vector.tensor_copy(out=v_sb[:, it, hf, :], in_=v_ps)

                # ---- B: attention, all 8 heads, own j columns ----
                outT = state.tile([128, KD, SJ], F32R, name=f"outT_{ly}", tag="outT",
                                  bufs=1)
                zparts = scr.tile([65, 8], F32, name=f"zp_{ly}", tag="zp", bufs=1)
                for hp in range(KD):  # head pair hp -> heads 2hp, 2hp+1
                    o_ps = [psum.tile([65, SJ], F32, name=f"o_ps_{ly}_{hp}_{hr}", tag="ps")
                            for hr in range(2)]
                    for it in range(IT):
                        l_ps = [psum.tile([128, SJ], F32,
                                          name=f"l_ps_{ly}_{hp}_{it}_{hr}", tag="ps")
                                for hr in range(2)]
                        for hr in range(2):
                            nc.tensor.matmul(l_ps[hr], lhsT=id_sb,
                                             rhs=maskp_sb[:, it, :],
                                             start=True, stop=False)
                        for hr in range(2):
                            pb = 64 * hr
                            nc.tensor.matmul(l_ps[hr],
                                             lhsT=qT[pb:pb + 64, hp,
                                                     it * 128:(it + 1) * 128],
                                             rhs=kT[pb:pb + 64, hp, :],
                                             start=False, stop=True)
                        for hr in range(2):
                            h_abs = 2 * hp + hr
                            e = scr.tile([128, SJ], F32R,
                                         name=f"e_{ly}_{hp}_{it}_{hr}", tag="e", bufs=3)
                            nc.scalar.activation(out=e, in_=l_ps[hr], func=AF.Exp,
                                                 scale=0.125)
                            nc.tensor.matmul(
                                o_ps[hr],
                                lhsT=v_sb[:, it, h_abs // 4,
                                          65 * (h_abs % 4):65 * (h_abs % 4) + 65],
                                rhs=e, start=(it == 0), stop=(it == IT - 1))
                    for hr in range(2):
                        h_abs = 2 * hp + hr
                        nc.vector.reduce_sum(out=zparts[64:65, h_abs:h_abs + 1],
                                             in_=o_ps[hr][64:65, :], axis=AX.X)
                        nc.scalar.activation(out=outT[64 * hr:64 * hr + 64, hp, :],
                                             in_=o_ps[hr][0:64, :], func=AF.Identity)

                # ---- Z exchange (32B AllReduce) + normalize outT ----
                ccz_in = dram.tile([1, 8], F32, name=f"ccz_in_{ly}", tag=f"ccz_in_{ly}")
                ccz_out = dram.tile([1, 8], F32, name=f"ccz_out_{ly}", tag=f"ccz_out_{ly}")
                nc.sync.dma_start(out=ccz_in, in_=zparts[64:65, :])
                nc.gpsimd.collective_compute("AllReduce", OP.add, replica_groups=GROUPS,
                                             ins=[ccz_in.opt()], outs=[ccz_out.opt()])
                z8 = scr.tile([8, 1], F32, name=f"z8_{ly}", tag="z8", bufs=1)
                nc.sync.dma_start(out=z8, in_=bass.AP(tensor=ccz_out.tensor,
                                                      offset=ccz_out.offset,
                                                      ap=[[1, 8], [1, 1]]))
                z8i = scr.tile([8, 1], F32, name=f"z8i_{ly}", tag="z8i", bufs=1)
                nc.vector.reciprocal(out=z8i, in_=z8)
                dg8 = scr.tile([8, 8], F32R, name=f"dg8_{ly}", tag="dg8", bufs=1)
                nc.vector.tensor_scalar(out=dg8, in0=id8_sb, scalar1=z8i,
                                        scalar2=None, op0=OP.mult)
                zps = psum.tile([128, 8], F32, name=f"zps_{ly}", tag="ps")
                nc.tensor.matmul(zps, lhsT=selp_sb, rhs=dg8, start=True, stop=True)
                zinv = scr.tile([128, KD], F32, name=f"zinv_{ly}", tag="zinv", bufs=1)
                nc.vector.reduce_sum(out=zinv,
                                     in_=zps.rearrange("p (m t) -> p m t", t=2),
                                     axis=AX.X)
                for m in range(KD):
                    nc.vector.tensor_scalar(out=outT[:, m, :], in0=_rd(outT[:, m, :]),
                                            scalar1=zinv[:, m:m + 1], scalar2=None,
                                            op0=OP.mult)

                # ---- C: out-projection + residual (bias folded) ----
                z1 = state.tile([128, KD, SJ], F32R, name=f"z1_{ly}", tag="qz", bufs=1)
                for dt_ in range(KD):
                    ap_ps = psum.tile([128, SJ], F32, name=f"ap_ps_{ly}_{dt_}", tag="ps")
                    for k in range(KD):
                        nc.tensor.matmul(ap_ps, lhsT=wo_sb[:, k, dt_ * 128:(dt_ + 1) * 128],
                                         rhs=outT[:, k, :],
                                         start=(k == 0), stop=(k == KD - 1))
                    nc.vector.scalar_tensor_tensor(
                        out=z1[:, dt_, :], in0=ap_ps, scalar=bo_sb[:, dt_:dt_ + 1],
                        in1=_rd(h_cur[:, dt_, 0:SJ]), op0=OP.add, op1=OP.add)

                # ---- D: LN1 ----
                h1 = state.tile([128, KD, SJ], F32R, name=f"h1_{ly}", tag="h1", bufs=1)
                layernorm(z1, g1_sb, be1_sb, f"h1_{ly}", h1)

                # ---- E: FFN (full DFF, own j-half), w2 streamed ----
                z2 = state.tile([128, KD, SJ], F32R, name=f"z2_{ly}", tag="qz", bufs=1)
                g_ps = [psum.tile([128, SJ], F32, name=f"g_ps_{ly}_{d}", tag="ps")
                        for d in range(KD)]
                for ft in range(KF):
                    w2c = scr.tile([128, D], F32R, name=f"w2c_{ly}_{ft}", tag="w2c", bufs=3)
                    nc.sync.dma_start(out=w2c, in_=w2r[:, ft, :])
                    f_ps = psum.tile([128, SJ], F32, name=f"f_ps_{ly}_{ft}", tag="ps")
                    for k in range(KD):
                        nc.tensor.matmul(f_ps, lhsT=w1_sb[:, k, ft * 128:(ft + 1) * 128],
                                         rhs=h1[:, k, :],
                                         start=(k == 0), stop=(k == KD - 1))
                    fr = scr.tile([128, SJ], F32R, name=f"fr_{ly}_{ft}", tag="fr", bufs=2)
                    nc.scalar.activation(out=fr, in_=f_ps, func=AF.Relu,
                                         bias=b1_sb[:, ft:ft + 1])
                    for d in range(KD):
                        nc.tensor.matmul(g_ps[d], lhsT=w2c[:, d * 128:(d + 1) * 128],
                                         rhs=fr, start=(ft == 0), stop=(ft == KF - 1))
                for d in range(KD):
                    nc.vector.scalar_tensor_tensor(
                        out=z2[:, d, :], in0=g_ps[d], scalar=b2_sb[:, d:d + 1],
                        in1=_rd(h1[:, d, :]), op0=OP.add, op1=OP.add)

                # ---- F: LN2 -> exchange halves (or final output) ----
                # AllReduce(own) gives own+peer; peer half = sum - own (1 ulp).
                if ly == 0:
                    h_next = state.tile([128, KD, S], F32R, name=f"h_{ly + 1}",
                                        tag="h", bufs=2)
                    layernorm(z2, g2_sb, be2_sb, f"hs_{ly}", h_next[:, :, 0:SJ])
                    ccs_in = dram.tile([D, SJ], F32, name=f"ccs_in_{ly}",
                                       tag=f"ccs_in_{ly}")
                    ccs_out = dram.tile([D, SJ], F32, name=f"ccs_out_{ly}",
                                        tag=f"ccs_out_{ly}")
                    nc.sync.dma_start(out=ccs_in.rearrange("(k p) s -> p k s", p=128),
                                      in_=_rd(h_next[:, :, 0:SJ]))
                    nc.gpsimd.collective_compute("AllReduce", OP.add,
                                                 replica_groups=GROUPS,
                                                 ins=[ccs_in.opt()], outs=[ccs_out.opt()])
                    ccsum = state.tile([128, KD, SJ], F32, name=f"ccsum_{ly}",
                                       tag="ccs", bufs=1)
                    nc.sync.dma_start(out=ccsum,
                                      in_=ccs_out.rearrange("(k p) s -> p k s", p=128))
                    for k in range(KD):
                        nc.vector.tensor_tensor(out=h_next[:, k, SJ:S],
                                                in0=ccsum[:, k, :],
                                                in1=_rd(h_next[:, k, 0:SJ]),
                                                op=OP.subtract)
                    h_cur = h_next
                else:
                    hstage = state.tile([128, KD, SJ], F32R, name=f"hs_{ly}", tag="ccs",
                                        bufs=1)
                    layernorm(z2, g2_sb, be2_sb, f"hs_{ly}", hstage)
                    nc.sync.dma_start(out=hout.rearrange("(k p) s -> p k s", p=128),
                                      in_=_rd(hstage))

    nc.compile()
    return nc


_CACHE = {}


def _prep_inputs(x, mask, Wq, bq, Wk, bk, Wv, bv, Wo, bo, W1, b1, W2, b2,
                 g1, be1, g2, be2):
    f32 = np.float32
    x = np.asarray(x, f32)
    mask = np.asarray(mask, f32)
    ident = np.eye(128, dtype=ml_dtypes.bfloat16)

    Wv = np.asarray(Wv, f32)
    bv = np.asarray(bv, f32)
    wva = np.zeros((D, VA), f32)
    bva = np.zeros((1, VA), f32)
    for h in range(H):
        wva[:, 65 * h:65 * h + 64] = Wv[:, 64 * h:64 * h + 64]
        bva[0, 65 * h:65 * h + 64] = bv[64 * h:64 * h + 64]
        bva[0, 65 * h + 64] = 1.0

    def pp(v, cols):
        return np.ascontiguousarray(np.asarray(v, f32).reshape(cols, 128).T)

    selp = np.zeros((H, 128), f32)
    for h in range(H):
        selp[h, (h % 2) * 64:(h % 2) * 64 + 64] = 1.0

    common = {
        "id8": np.eye(H, dtype=f32),
        "selp": selp,
        "ident": ident,
        "wq": np.ascontiguousarray(np.asarray(Wq, f32)),
        "wk": np.ascontiguousarray(np.asarray(Wk, f32)),
        "wva": wva,
        "wo": np.ascontiguousarray(np.asarray(Wo, f32)),
        "w1": np.ascontiguousarray(np.asarray(W1, f32)),
        "w2": np.ascontiguousarray(np.asarray(W2, f32)),
        "bq": pp(bq, KD),
        "bk": pp(bk, KD),
        "bva": bva,
        "bo": pp(bo, KD),
        "b1": pp(b1, KF),
        "b2": pp(b2, KD),
        "g1": pp(g1, KD),
        "be1": pp(be1, KD),
        "g2": pp(g2, KD),
        "be2": pp(be2, KD),
    }
    in_maps = []
    for c in range(N_CORES):
        b, r = c // 2, c % 2
        js = slice(r * SJ, (r + 1) * SJ)
        ps = slice((1 - r) * SJ, (2 - r) * SJ)
        # local token order: own half first (both in h columns and mask rows)
        xb = x[b].T
        m = dict(common)
        m["xT"] = np.ascontiguousarray(np.concatenate([xb[:, js], xb[:, ps]], axis=1))
        mrows = np.concatenate([mask[b][js], mask[b][ps]], axis=0)
        m["maskp"] = np.ascontiguousarray(mrows[:, js] * NEG).astype(ml_dtypes.bfloat16)
        in_maps.append(m)
    return in_maps


def get_nc():
    if "nc" not in _CACHE:
        _CACHE["nc"] = build()
    return _CACHE["nc"]


def run(in_maps, **kw):
    nc = get_nc()
    return run_bass_kernel_spmd(nc, in_maps, core_ids=list(range(N_CORES)), **kw)


def kernel(**inputs):
    in_maps = _prep_inputs(**inputs)
    res = run(in_maps)
    out = np.empty((B, S, D), np.float32)
    for c in range(N_CORES):
        b, r = c // 2, c % 2
        out[b, r * SJ:(r + 1) * SJ, :] = res.results[c]["hout"].T
    return out


# revision 29
# speedup vs baseline: 1.6589x; 1.0385x over previous
"""Trainium2 Bass kernel for nn_Encoders (2-layer shared-weight transformer encoder).

Sharding (v3): 8 cores; pair (2b, 2b+1) handles batch b.  Within a pair the
split is along the attention *output* token axis j (the reference's unusual
attention contracts over queries i: out[j,d] = sum_i attn[i,j] v[i,d]):

  - each core computes q and v for ALL tokens/heads (small duplication),
    k only for its own j-half,
  - E = exp((qk^T + mask*NEG)/8) for its own j columns, all heads,
  - attention output, out-projection, residual+LN1, full-DFF FFN,
    residual+LN2 for its own j-half only -- NO partial-sum collectives.

Cross-core data: the joint-softmax denominator Z (per head) is summed with a
32-byte AllReduce, and the layer output h is AllGathered (1MB) at the layer
boundary; the final layer outputs each core's own half directly.

Everything stays in transposed layout [feature, token]; LN stats via all-ones
matmul (broadcast sums).  Matmul inputs are float32r (full-rate PE, fp32
storage): producers write f32r, DVE/ACT consumers read via bitcast to f32.
A ones-column appended to v (via host-built wv_aug/bv_aug) makes the attnV
matmul emit per-column E sums for free -> Z without activation accumulators.
"""

import sys

sys.path.insert(0, "/opt/trn_rl_repo")

import numpy as np
import ml_dtypes

import concourse.bass as bass
import concourse.mybir as mybir
import concourse.tile as tile
from concourse import bacc
from concourse.bass_utils import run_bass_kernel_spmd

F32 = mybir.dt.float32
F32R = mybir.dt.float32r
BF16 = mybir.dt.bfloat16
AF = mybir.ActivationFunctionType
OP = mybir.AluOpType
AX = mybir.AxisListType

B, S, D, H, DFF = 4, 1024, 512, 8, 2048
DEPTH = D // H  # 64
NEG = -1.0e9
EPS = 1e-9
N_CORES = 8
GROUPS = [[0, 1], [2, 3], [4, 5], [6, 7]]

SJ = S // 2       # own token half: 512
KD = D // 128     # 4 k-tiles over D
IT = S // 128     # 8 i tiles
KF = DFF // 128   # 16 dff tiles
VA = 2 * (4 * 65)  # v augmented with a ones column per head: 2 halves x 260


def _rd(ap):
    return ap.bitcast(F32)


def build():
    nc = bacc.Bacc("TRN2", target_bir_lowering=False, debug=False,
                   num_devices=N_CORES)

    def din(name, shape, dt=F32):
        return nc.dram_tensor(name, shape, dt, kind="ExternalInput").ap()

    xT = din("xT", [D, S], F32R)
    xb = din("xb", [D, S], BF16)
    maskp = din("maskp", [S, SJ], BF16)         # mask[:, own j] * (-1e9)
    ident = din("ident", [128, 128], BF16)
    wq = din("wq", [D, D], BF16)
    wk = din("wk", [D, D], BF16)
    wva = din("wva", [D, VA], BF16)             # v weights with ones-cols
    wo = din("wo", [D, D], BF16)
    w1 = din("w1", [D, DFF], BF16)
    w2 = din("w2", [DFF, D], BF16)
    bq = din("bq", [128, KD])
    bk = din("bk", [128, KD])
    bva = din("bva", [1, VA], BF16)
    bo = din("bo", [128, KD])
    b1 = din("b1", [128, KF])
    b2 = din("b2", [128, KD])
    g1 = din("g1", [128, KD])
    be1 = din("be1", [128, KD])
    g2 = din("g2", [128, KD])
    be2 = din("be2", [128, KD])
    id8 = din("id8", [8, 8])
    selp = din("selp", [8, 128], F32R)
    hout = nc.dram_tensor("hout", [D, SJ], F32, kind="ExternalOutput").ap()

    with tile.TileContext(nc) as tc:
        with (
            tc.tile_pool(name="const", bufs=1) as const,
            tc.tile_pool(name="state", bufs=1) as state,
            tc.tile_pool(name="scr", bufs=1) as scr,
            tc.tile_pool(name="psum", bufs=8, space="PSUM") as psum,
            tc.tile_pool(name="dram", bufs=1, space="DRAM") as dram,
        ):
            def loadw(name, src, kt, m, dt=BF16):
                t = const.tile([128, kt, m], dt, name=name, tag=name)
                nc.sync.dma_start(out=t, in_=src.rearrange("(k p) m -> p k m", p=128))
                return t

            def loadsm(name, src, dt=F32):
                t = const.tile(list(src.shape), dt, name=name, tag=name)
                nc.sync.dma_start(out=t, in_=src)
                return t

            # load order matters: the first qkv matmuls need x + wq/wk/wva
            h_cur = state.tile([128, KD, S], F32R, name="h0", tag="h", bufs=2)
            nc.sync.dma_start(out=h_cur, in_=xT.rearrange("(k p) s -> p k s", p=128))
            hb = state.tile([128, KD, S], BF16, name="hb0", tag="hb", bufs=2)
            nc.sync.dma_start(out=hb, in_=xb.rearrange("(k p) s -> p k s", p=128))
            wq_sb = loadw("wq_sb", wq, KD, D)
            wk_sb = loadw("wk_sb", wk, KD, D)
            wva_sb = loadw("wva_sb", wva, KD, VA)
            bq_sb = loadsm("bq_sb", bq)
            bk_sb = loadsm("bk_sb", bk)
            bva_sb = loadsm("bva_sb", bva, BF16)
            ones_sq = const.tile([128, 128], F32R, name="ones_sq", tag="ones_sq")
            nc.vector.memset(_rd(ones_sq), 1.0)
            ones_bf = const.tile([1, 128], BF16, name="ones_bf", tag="ones_bf")
            nc.vector.memset(ones_bf, 1.0)
            maskp_sb = const.tile([128, IT, SJ], BF16, name="maskp_sb", tag="maskp_sb")
            nc.sync.dma_start(out=maskp_sb, in_=maskp.rearrange("(i p) j -> p i j", p=128))
            id_sb = const.tile([128, 128], BF16, name="id_sb", tag="id_sb")
            nc.sync.dma_start(out=id_sb, in_=ident)
            wo_sb = loadw("wo_sb", wo, KD, D)
            w1_sb = loadw("w1_sb", w1, KD, DFF)
            bo_sb = loadsm("bo_sb", bo)
            b1_sb = loadsm("b1_sb", b1)
            b2_sb = loadsm("b2_sb", b2)
            g1_sb = loadsm("g1_sb", g1)
            be1_sb = loadsm("be1_sb", be1)
            g2_sb = loadsm("g2_sb", g2)
            be2_sb = loadsm("be2_sb", be2)
            eps_sb = const.tile([128, 1], F32, name="eps_sb", tag="eps_sb")
            nc.vector.memset(eps_sb, EPS)
            id8_sb = loadsm("id8_sb", id8)
            selp_sb = loadsm("selp_sb", selp, F32R)
            w2r = w2.rearrange("(k p) m -> p k m", p=128)

            def layernorm(z, g_sb, be_sb, out_name, out_tile):
                """z: f32r [128, KD, SJ]; writes normalized f32r into out_tile."""
                s1 = psum.tile([128, SJ], F32, name=f"s1_{out_name}", tag="ps")
                s2 = psum.tile([128, SJ], F32, name=f"s2_{out_name}", tag="ps")
                for k in range(KD):
                    sqc = scr.tile([128, SJ], F32R, name=f"sq_{out_name}_{k}",
                                   tag="e", bufs=4)
                    nc.scalar.activation(out=sqc, in_=_rd(z[:, k, :]), func=AF.Square)
                    nc.tensor.matmul(s1, lhsT=ones_sq, rhs=z[:, k, :],
                                     start=(k == 0), stop=(k == KD - 1))
                    nc.tensor.matmul(s2, lhsT=ones_sq, rhs=sqc,
                                     start=(k == 0), stop=(k == KD - 1))
                mean = scr.tile([128, SJ], F32, name=f"mean_{out_name}", tag="mean", bufs=1)
                rstd = scr.tile([128, SJ], F32, name=f"rstd_{out_name}", tag="rstd", bufs=1)
                nc.vector.tensor_scalar(out=mean, in0=s1, scalar1=1.0 / D,
                                        scalar2=None, op0=OP.mult)
                msq = scr.tile([128, SJ], F32, name=f"msq_{out_name}", tag="e", bufs=4)
                nc.vector.tensor_tensor(out=msq, in0=mean, in1=mean, op=OP.mult)
                var = scr.tile([128, SJ], F32, name=f"var_{out_name}", tag="e", bufs=4)
                nc.vector.scalar_tensor_tensor(out=var, in0=s2, scalar=1.0 / D,
                                               in1=msq, op0=OP.mult, op1=OP.subtract)
                nc.scalar.activation(out=var, in_=var, func=AF.Sqrt, bias=eps_sb[:, 0:1])
                nc.vector.reciprocal(out=rstd, in_=var)
                for k in range(KD):
                    t = scr.tile([128, SJ], F32, name=f"t_{out_name}_{k}",
                                 tag="e", bufs=4)
                    nc.vector.tensor_tensor(out=t, in0=_rd(z[:, k, :]), in1=mean,
                                            op=OP.subtract)
                    nc.vector.tensor_tensor(out=t, in0=t, in1=rstd, op=OP.mult)
                    nc.vector.tensor_scalar(out=out_tile[:, k, :], in0=t,
                                            scalar1=g_sb[:, k:k + 1],
                                            scalar2=be_sb[:, k:k + 1],
                                            op0=OP.mult, op1=OP.add)

            for ly in range(2):
                # ---- A: projections. q,v for all tokens; k for own half ----
                qT = state.tile([128, KD, S], BF16, name=f"qT_{ly}", tag="qT", bufs=1)
                kT = state.tile([128, KD, SJ], BF16, name=f"kT_{ly}", tag="kT", bufs=1)
                for m in range(KD):
                    k_ps = psum.tile([128, SJ], F32, name=f"k_ps_{ly}_{m}", tag="ps")
                    for k in range(KD):
                        nc.tensor.matmul(k_ps, lhsT=wk_sb[:, k, m * 128:(m + 1) * 128],
                                         rhs=hb[:, k, 0:SJ],
                                         start=(k == 0), stop=(k == KD - 1))
                    nc.scalar.activation(out=kT[:, m, :], in_=k_ps,
                                         func=AF.Identity, bias=bk_sb[:, m:m + 1])
                for m in range(KD):
                    for sc in range(2):
                        ssl = slice(sc * SJ, (sc + 1) * SJ)
                        q_ps = psum.tile([128, SJ], F32, name=f"q_ps_{ly}_{m}_{sc}", tag="ps")
                        for k in range(KD):
                            nc.tensor.matmul(q_ps, lhsT=wq_sb[:, k, m * 128:(m + 1) * 128],
                                             rhs=hb[:, k, ssl],
                                             start=(k == 0), stop=(k == KD - 1))
                        nc.scalar.activation(out=qT[:, m, ssl], in_=q_ps,
                                             func=AF.Identity, bias=bq_sb[:, m:m + 1])
                v_sb = state.tile([128, IT, 2, 260], BF16, name=f"v_{ly}", tag="v", bufs=1)
                for it in range(IT):
                    for hf in range(2):
                        v_ps = psum.tile([128, 260], F32, name=f"v_ps_{ly}_{it}_{hf}",
                                         tag="ps")
                        for k in range(KD):
                            nc.tensor.matmul(v_ps,
                                             lhsT=hb[:, k, it * 128:(it + 1) * 128],
                                             rhs=wva_sb[:, k, hf * 260:(hf + 1) * 260],
                                             start=(k == 0), stop=False)
                        nc.tensor.matmul(v_ps, lhsT=ones_bf,
                                         rhs=bva_sb[0:1, hf * 260:(hf + 1) * 260],
                                         start=False, stop=True)
                        nc.vector.tensor_copy(out=v_sb[:, it, hf, :], in_=v_ps)

                # ---- B: attention, all 8 heads, own j columns ----
                outT = state.tile([128, KD, SJ], BF16, name=f"outT_{ly}", tag="outT",
                                  bufs=1)
                zparts = scr.tile([65, 8], F32, name=f"zp_{ly}", tag="zp", bufs=1)
                for hp in range(KD):  # head pair hp -> heads 2hp, 2hp+1
                    o_ps = [psum.tile([65, SJ], F32, name=f"o_ps_{ly}_{hp}_{hr}", tag="ps")
                            for hr in range(2)]
                    for it in range(IT):
                        l_ps = [psum.tile([128, SJ], F32,
                                          name=f"l_ps_{ly}_{hp}_{it}_{hr}", tag="ps")
                                for hr in range(2)]
                        for hr in range(2):
                            nc.tensor.matmul(l_ps[hr], lhsT=id_sb,
                                             rhs=maskp_sb[:, it, :],
                                             start=True, stop=False)
                        for hr in range(2):
                            pb = 64 * hr
                            nc.tensor.matmul(l_ps[hr],
                                             lhsT=qT[pb:pb + 64, hp,
                                                     it * 128:(it + 1) * 128],
                                             rhs=kT[pb:pb + 64, hp, :],
                                             start=False, stop=True)
                        for hr in range(2):
                            h_abs = 2 * hp + hr
                            e = scr.tile([128, SJ], BF16,
                                         name=f"e_{ly}_{hp}_{it}_{hr}", tag="e", bufs=4)
                            nc.scalar.activation(out=e, in_=l_ps[hr], func=AF.Exp,
                                                 scale=0.125)
                            nc.tensor.matmul(
                                o_ps[hr],
                                lhsT=v_sb[:, it, h_abs // 4,
                                          65 * (h_abs % 4):65 * (h_abs % 4) + 65],
                                rhs=e, start=(it == 0), stop=(it == IT - 1))
                    for hr in range(2):
                        h_abs = 2 * hp + hr
                        nc.vector.reduce_sum(out=zparts[64:65, h_abs:h_abs + 1],
                                             in_=o_ps[hr][64:65, :], axis=AX.X)
                        nc.scalar.activation(out=outT[64 * hr:64 * hr + 64, hp, :],
                                             in_=o_ps[hr][0:64, :], func=AF.Identity)

                # ---- Z exchange (32B AllReduce) + normalize outT ----
                ccz_in = dram.tile([1, 8], F32, name=f"ccz_in_{ly}", tag=f"ccz_in_{ly}")
                ccz_out = dram.tile([1, 8], F32, name=f"ccz_out_{ly}", tag=f"ccz_out_{ly}")
                nc.sync.dma_start(out=ccz_in, in_=zparts[64:65, :])
                nc.gpsimd.collective_compute("AllReduce", OP.add, replica_groups=GROUPS,
                                             ins=[ccz_in.opt()], outs=[ccz_out.opt()])
                z8 = scr.tile([8, 1], F32, name=f"z8_{ly}", tag="z8", bufs=1)
                nc.sync.dma_start(out=z8, in_=bass.AP(tensor=ccz_out.tensor,
                                                      offset=ccz_out.offset,
                                                      ap=[[1, 8], [1, 1]]))
                z8i = scr.tile([8, 1], F32, name=f"z8i_{ly}", tag="z8i", bufs=1)
                nc.vector.reciprocal(out=z8i, in_=z8)
                dg8 = scr.tile([8, 8], F32R, name=f"dg8_{ly}", tag="dg8", bufs=1)
                nc.vector.tensor_scalar(out=dg8, in0=id8_sb, scalar1=z8i,
                                        scalar2=None, op0=OP.mult)
                zps = psum.tile([128, 8], F32, name=f"zps_{ly}", tag="ps")
                nc.tensor.matmul(zps, lhsT=selp_sb, rhs=dg8, start=True, stop=True)
                zinv = scr.tile([128, KD], F32, name=f"zinv_{ly}", tag="zinv", bufs=1)
                nc.vector.reduce_sum(out=zinv,
                                     in_=zps.rearrange("p (m t) -> p m t", t=2),
                                     axis=AX.X)
                for m in range(KD):
                    nc.vector.tensor_scalar(out=outT[:, m, :], in0=outT[:, m, :],
                                            scalar1=zinv[:, m:m + 1], scalar2=None,
                                            op0=OP.mult)

                # ---- C: out-projection + residual (bias folded) ----
                z1 = state.tile([128, KD, SJ], F32R, name=f"z1_{ly}", tag="qz", bufs=1)
                for dt_ in range(KD):
                    ap_ps = psum.tile([128, SJ], F32, name=f"ap_ps_{ly}_{dt_}", tag="ps")
                    for k in range(KD):
                        nc.tensor.matmul(ap_ps, lhsT=wo_sb[:, k, dt_ * 128:(dt_ + 1) * 128],
                                         rhs=outT[:, k, :],
                                         start=(k == 0), stop=(k == KD - 1))
                    nc.vector.scalar_tensor_tensor(
                        out=z1[:, dt_, :], in0=ap_ps, scalar=bo_sb[:, dt_:dt_ + 1],
                        in1=_rd(h_cur[:, dt_, 0:SJ]), op0=OP.add, op1=OP.add)

                # ---- D: LN1 ----
                h1 = state.tile([128, KD, SJ], F32R, name=f"h1_{ly}", tag="h1", bufs=1)
                layernorm(z1, g1_sb, be1_sb, f"h1_{ly}", h1)
                h1b = state.tile([128, KD, SJ], BF16, name=f"h1b_{ly}", tag="h1b", bufs=1)
                for k in range(KD):
                    nc.vector.tensor_copy(out=h1b[:, k, :], in_=_rd(h1[:, k, :]))

                # ---- E: FFN (full DFF, own j-half), w2 streamed ----
                z2 = state.tile([128, KD, SJ], F32R, name=f"z2_{ly}", tag="qz", bufs=1)
                g_ps = [psum.tile([128, SJ], F32, name=f"g_ps_{ly}_{d}", tag="ps")
                        for d in range(KD)]
                for ft in range(KF):
                    w2c = scr.tile([128, D], BF16, name=f"w2c_{ly}_{ft}", tag="w2c", bufs=4)
                    nc.sync.dma_start(out=w2c, in_=w2r[:, ft, :])
                    f_ps = psum.tile([128, SJ], F32, name=f"f_ps_{ly}_{ft}", tag="ps")
                    for k in range(KD):
                        nc.tensor.matmul(f_ps, lhsT=w1_sb[:, k, ft * 128:(ft + 1) * 128],
                                         rhs=h1b[:, k, :],
                                         start=(k == 0), stop=(k == KD - 1))
                    fr = scr.tile([128, SJ], BF16, name=f"fr_{ly}_{ft}", tag="fr", bufs=3)
                    nc.scalar.activation(out=fr, in_=f_ps, func=AF.Relu,
                                         bias=b1_sb[:, ft:ft + 1])
                    for d in range(KD):
                        nc.tensor.matmul(g_ps[d], lhsT=w2c[:, d * 128:(d + 1) * 128],
                                         rhs=fr, start=(ft == 0), stop=(ft == KF - 1))
                for d in range(KD):
                    nc.vector.scalar_tensor_tensor(
                        out=z2[:, d, :], in0=g_ps[d], scalar=b2_sb[:, d:d + 1],
                        in1=_rd(h1[:, d, :]), op0=OP.add, op1=OP.add)

                # ---- F: LN2 -> exchange halves (or final output) ----
                # AllReduce(own) gives own+peer; peer half = sum - own (1 ulp).
                if ly == 0:
                    h_next = state.tile([128, KD, S], F32R, name=f"h_{ly + 1}",
                                        tag="h", bufs=2)
                    layernorm(z2, g2_sb, be2_sb, f"hs_{ly}", h_next[:, :, 0:SJ])
                    ccs_in = dram.tile([D, SJ], F32, name=f"ccs_in_{ly}",
                                       tag=f"ccs_in_{ly}")
                    ccs_out = dram.tile([D, SJ], F32, name=f"ccs_out_{ly}",
                                        tag=f"ccs_out_{ly}")
                    nc.sync.dma_start(out=ccs_in.rearrange("(k p) s -> p k s", p=128),
                                      in_=_rd(h_next[:, :, 0:SJ]))
                    nc.gpsimd.collective_compute("AllReduce", OP.add,
                                                 replica_groups=GROUPS,
                                                 ins=[ccs_in.opt()], outs=[ccs_out.opt()])
                    ccsum = state.tile([128, KD, SJ], F32, name=f"ccsum_{ly}",
                                       tag="ccs", bufs=1)
                    nc.sync.dma_start(out=ccsum,
                                      in_=ccs_out.rearrange("(k p) s -> p k s", p=128))
                    hb_next = state.tile([128, KD, S], BF16, name=f"hb_{ly + 1}",
                                         tag="hb", bufs=2)
                    for k in range(KD):
                        nc.vector.tensor_tensor(out=h_next[:, k, SJ:S],
                                                in0=ccsum[:, k, :],
                                                in1=_rd(h_next[:, k, 0:SJ]),
                                                op=OP.subtract)
                    for k in range(KD):
                        nc.vector.tensor_copy(out=hb_next[:, k, 0:SJ],
                                              in_=_rd(h_next[:, k, 0:SJ]))
                        nc.vector.tensor_copy(out=hb_next[:, k, SJ:S],
                                              in_=_rd(h_next[:, k, SJ:S]))
                    h_cur = h_next
                    hb = hb_next
                else:
                    hstage = state.tile([128, KD, SJ], F32R, name=f"hs_{ly}", tag="ccs",
                                        bufs=1)
                    layernorm(z2, g2_sb, be2_sb, f"hs_{ly}", hstage)
                    nc.sync.dma_start(out=hout.rearrange("(k p) s -> p k s", p=128),
                                      in_=_rd(hstage))

    nc.compile()
    return nc


_CACHE = {}


def _prep_inputs(x, mask, Wq, bq, Wk, bk, Wv, bv, Wo, bo, W1, b1, W2, b2,
                 g1, be1, g2, be2):
    f32 = np.float32
    x = np.asarray(x, f32)
    mask = np.asarray(mask, f32)
    ident = np.eye(128, dtype=ml_dtypes.bfloat16)

    Wv = np.asarray(Wv, f32)
    bv = np.asarray(bv, f32)
    wva = np.zeros((D, VA), f32)
    bva = np.zeros((1, VA), f32)
    for h in range(H):
        wva[:, 65 * h:65 * h + 64] = Wv[:, 64 * h:64 * h + 64]
        bva[0, 65 * h:65 * h + 64] = bv[64 * h:64 * h + 64]
        bva[0, 65 * h + 64] = 1.0

    def pp(v, cols):
        return np.ascontiguousarray(np.asarray(v, f32).reshape(cols, 128).T)

    selp = np.zeros((H, 128), f32)
    for h in range(H):
        selp[h, (h % 2) * 64:(h % 2) * 64 + 64] = 1.0

    bf16 = ml_dtypes.bfloat16
    common = {
        "id8": np.eye(H, dtype=f32),
        "selp": selp,
        "ident": ident,
        "wq": np.asarray(Wq, f32).astype(bf16),
        "wk": np.asarray(Wk, f32).astype(bf16),
        "wva": wva.astype(bf16),
        "wo": np.asarray(Wo, f32).astype(bf16),
        "w1": np.asarray(W1, f32).astype(bf16),
        "w2": np.asarray(W2, f32).astype(bf16),
        "bq": pp(bq, KD),
        "bk": pp(bk, KD),
        "bva": bva.astype(bf16),
        "bo": pp(bo, KD),
        "b1": pp(b1, KF),
        "b2": pp(b2, KD),
        "g1": pp(g1, KD),
        "be1": pp(be1, KD),
        "g2": pp(g2, KD),
        "be2": pp(be2, KD),
    }
    in_maps = []
    for c in range(N_CORES):
        b, r = c // 2, c % 2
        js = slice(r * SJ, (r + 1) * SJ)
        ps = slice((1 - r) * SJ, (2 - r) * SJ)
        # local token order: own half first (both in h columns and mask rows)
        xb = x[b].T
        m = dict(common)
        xtl = np.ascontiguousarray(np.concatenate([xb[:, js], xb[:, ps]], axis=1))
        m["xT"] = xtl
        m["xb"] = xtl.astype(bf16)
        mrows = np.concatenate([mask[b][js], mask[b][ps]], axis=0)
        m["maskp"] = np.ascontiguousarray(mrows[:, js] * NEG).astype(ml_dtypes.bfloat16)
        in_maps.append(m)
    return in_maps


def get_nc():
    if "nc" not in _CACHE:
        _CACHE["nc"] = build()
    return _CACHE["nc"]


def run(in_maps, **kw):
    nc = get_nc()
    return run_bass_kernel_spmd(nc, in_maps, core_ids=list(range(N_CORES)), **kw)


def kernel(**inputs):
    in_maps = _prep_inputs(**inputs)
    res = run(in_maps)
    out = np.empty((B, S, D), np.float32)
    for c in range(N_CORES):
        b, r = c // 2, c % 2
        out[b, r * SJ:(r + 1) * SJ, :] = res.results[c]["hout"].T
    return out
